# revision 1
# baseline (speedup 1.0000x reference)
"""CTC-style loss (nn_CTCFormal) on 8 Trainium2 NeuronCores.

Pure data parallel over batch N=4096 -> 512 samples/core.

Host prep replicates the reference's (buggy) target padding to get per-sample
labels lab[n, 0:31] and skip masks, and lays out the per-sample label-class
log-prob rows [P, G, L, T] plus blank rows [P, G, T] for upload (the class
gather is index-only data movement; this environment's SWDGE gather/indirect
DMA ucode faults, so the gather is done during input layout instead).

Device (per core): plain DMAs load the rows, ACT computes exp, DVE runs the
blank-interleaved alpha recurrence with samples on partitions and the state
split into even (blank) / odd (label) lanes, ACT takes -log, and per-sample
losses are DMA'd out.  The host sums the 8x512 partials (the all-reduce of
the scalar loss).
"""

import numpy as np

T, N, C = 64, 4096, 128
L = 31           # labels per sample
NCORES = 8
NLOC = N // NCORES          # 512 samples per core
G = NLOC // 128             # 4 groups of 128 samples (partition dim)
P = 128

_BASS_CACHE = {}


def _build_bass():
    if "nc" in _BASS_CACHE:
        return _BASS_CACHE["nc"]

    import concourse.bacc as bacc
    import concourse.mybir as mybir
    from concourse.tile import TileContext

    f32 = mybir.dt.float32
    AF = mybir.ActivationFunctionType

    # Bacc (not raw Bass): its compile() splits multi-sem waits into event
    # semaphores — TRN2 instructions have a single sync-wait slot.
    nc = bacc.Bacc(trn_type="TRN2")
    xsl_d = nc.declare_dram_parameter("xsl", [P, G, L, T], f32, isOutput=False)
    blkl_d = nc.declare_dram_parameter("blkl", [P, G, T], f32, isOutput=False)
    skip_d = nc.declare_dram_parameter("skipm", [P, G, L], f32, isOutput=False)
    loss_d = nc.declare_dram_parameter("loss", [P, G], f32, isOutput=True)

    with TileContext(nc) as tc:
        with tc.tile_pool(name="main", bufs=1) as pool:
            skip_s = pool.tile([P, G, L], f32)
            nc.sync.dma_start(out=skip_s[:], in_=skip_d[:])

            blkl = pool.tile([P, G, T], f32)
            nc.sync.dma_start(out=blkl[:], in_=blkl_d[:])
            blk = pool.tile([P, G, T], f32)
            nc.scalar.activation(out=blk[:], in_=blkl[:], func=AF.Exp)

            # label-row log-probs, split per group so exp overlaps the DMAs
            ysl = pool.tile([P, G, L, T], f32)
            ys = pool.tile([P, G, L, T], f32)
            for g in range(G):
                nc.sync.dma_start(out=ysl[:, g], in_=xsl_d[:, g])
                nc.scalar.activation(out=ys[:, g], in_=ysl[:, g], func=AF.Exp)

            # alpha state: cols 0,1 stay zero; state s=0..62 lives in cols 2..64
            a = pool.tile([P, G, 65], f32)
            b = pool.tile([P, G, 65], f32)
            u = pool.tile([P, G, 63], f32)
            vv = pool.tile([P, G, L], f32)
            nc.vector.memset(a[:], 0.0)
            nc.vector.memset(b[:], 0.0)
            # alpha0[s=0] = blank[t=0]; alpha0[s=1] = ylab[t=0, j=0]
            # (on ACT: DVE TensorCopy's ISA struct only has one sync-wait slot,
            # and these copies need waits on both the memset and the exp)
            nc.scalar.copy(out=a[:, :, 2], in_=blk[:, :, 0])
            nc.scalar.copy(out=a[:, :, 3], in_=ys[:, :, 0, 0])

            cur, nxt = a, b
            for t in range(1, T):
                # u[s] = alpha[s] + alpha[s-1], s = 0..62
                nc.vector.tensor_add(
                    out=u[:], in0=cur[:, :, 2:65], in1=cur[:, :, 1:64]
                )
                # vv[j] = skip[j] * alpha[s-2] at odd s=2j+1 (alpha[2j-1] = col 1+2j)
                nc.vector.tensor_mul(
                    out=vv[:], in0=cur[:, :, 1:63:2], in1=skip_s[:]
                )
                nc.vector.tensor_add(
                    out=u[:, :, 1:63:2], in0=u[:, :, 1:63:2], in1=vv[:]
                )
                # even lanes (s=2j, col 2+2j): multiply by blank prob at t
                nc.vector.tensor_mul(
                    out=nxt[:, :, 2:65:2],
                    in0=u[:, :, 0:63:2],
                    in1=blk[:, :, t : t + 1].to_broadcast([P, G, 32]),
                )
                # odd lanes (s=2j+1, col 3+2j): multiply by label probs at t
                nc.vector.tensor_mul(
                    out=nxt[:, :, 3:65:2],
                    in0=u[:, :, 1:63:2],
                    in1=ys[:, :, :, t],
                )
                cur, nxt = nxt, cur

            # loss = -log(alpha_T[s=62] + alpha_T[s=61]) = cols 64, 63
            r = pool.tile([P, G], f32)
            nc.vector.tensor_add(out=r[:], in0=cur[:, :, 64], in1=cur[:, :, 63])
            lg = pool.tile([P, G], f32)
            nc.scalar.activation(out=lg[:], in_=r[:], func=AF.Ln)
            neg = pool.tile([P, G], f32)
            nc.scalar.mul(out=neg[:], in_=lg[:], mul=-1.0)
            nc.sync.dma_start(out=loss_d[:], in_=neg[:])

    # bass2jax's PJRT path serializes nc.m without calling finalize();
    # Bacc defers register allocation to compile() (run by finalize), so
    # finalize here or walrus sees unallocated registers.
    nc.finalize()
    _BASS_CACHE["nc"] = nc
    return nc


def host_prep(input, target, input_length, target_length):
    """Build the 8 per-core input maps."""
    inp = np.asarray(input, dtype=np.float32)
    target = np.asarray(target, dtype=np.int32)
    tl = np.asarray(target_length, dtype=np.int64)

    # reference's buggy padding: start_i = target_length[i-1] if i>0 else 0,
    # clamped like jax.lax.dynamic_slice
    starts = np.zeros(N, np.int64)
    starts[1:] = tl[: N - 1]
    starts = np.clip(starts, 0, len(target) - L)
    lab = target[starts[:, None] + np.arange(L)]  # [N, L] int32
    skipm = np.zeros((N, L), np.float32)
    skipm[:, 1:] = (lab[:, 1:] != lab[:, :-1]).astype(np.float32)

    x_nct = inp.transpose(1, 2, 0)  # [N, C, T] view
    # per-sample label rows [N, L, T] and blank rows [N, T]
    xs = np.take_along_axis(x_nct, lab[:, :, None].astype(np.int64), axis=1)
    blk = x_nct[:, 0, :]

    in_maps = []
    for core in range(NCORES):
        sl = slice(core * NLOC, (core + 1) * NLOC)
        xs_c = xs[sl].reshape(G, P, L, T).transpose(1, 0, 2, 3)
        blk_c = blk[sl].reshape(G, P, T).transpose(1, 0, 2)
        skip_c = skipm[sl].reshape(G, P, L).transpose(1, 0, 2)
        in_maps.append(
            {
                "xsl": np.ascontiguousarray(xs_c),
                "blkl": np.ascontiguousarray(blk_c),
                "skipm": np.ascontiguousarray(skip_c),
            }
        )
    return in_maps


def kernel(input, target, input_length, target_length):
    from concourse.bass_utils import run_bass_kernel_spmd

    nc = _build_bass()
    in_maps = host_prep(input, target, input_length, target_length)
    res = run_bass_kernel_spmd(nc, in_maps, list(range(NCORES)))
    total = 0.0
    for core in range(NCORES):
        total += float(np.asarray(res.results[core]["loss"], dtype=np.float64).sum())
    return np.float32(total)



# revision 3
# speedup vs baseline: 1.3570x; 1.3570x over previous
"""CTC-style loss (nn_CTCFormal) on 8 Trainium2 NeuronCores.

Pure data parallel over batch N=4096 -> 512 samples/core, laid out as
[P=128 partitions, G=4 groups].

Formulation: the alpha recurrence is rescaled by the per-step blank
probability.  With a~[t,s] = alpha[t,s] / prod_{tau<=t} y_blank(tau):

  even s (blanks):  a~[t,s] = a~[t-1,s] + a~[t-1,s-1]           (no multiply)
  odd  s (labels):  a~[t,s] = (a~[t-1,s] + a~[t-1,s-1]) * r[t,j]
                              + a~[t-1,s-2] * r2[t,j]
  loss = -( log(a~[T-1,S-1] + a~[T-1,S-2]) + sum_t x_blank[t] )

where r = exp(x_lab - x_blank) and r2 = z * r with z the CTC skip mask
(z=0 when lab[j]==lab[j-1]); r2 comes from exp() of a -1e30-masked copy.
The blank product becomes a log-space reduce_sum of raw blank logits, so
the blank multiply disappears from the inner loop entirely (4 DVE ops/step
instead of 5, and one of them is the only op that reads r).

The alpha DP is banded: at step t only states s in [max(0,2t-66),
min(62,2t+1)] are live (states outside can't be on any path from
(0,{0,1}) to (T-1,{S-1,S-2})), so ops slice just the live band (~53% of
the elements, exact).

Host prep replicates the reference's (buggy) target padding, gathers the
per-sample label-class logit rows (index-only data movement; this
environment's SWDGE gather ucode faults), subtracts the blank row, and
ships bf16 [P, T, G, L] time-major tensors so each step's r slice is
contiguous.

Device (per core): chunked DMAs + ACT exp produce r/r2 overlapped with
the DVE recurrence; final -log and blank reduce_sum close it out.  The
host sums the 8x512 partials (the all-reduce of the scalar loss sum).
"""

import numpy as np

T, N, C = 64, 4096, 128
L = 31           # labels per sample
S = 2 * L + 1    # 63 padded states
NCORES = 8
NLOC = N // NCORES          # 512 samples per core
G = NLOC // 128             # 4 groups of 128 samples (partition dim)
P = 128
TCH = 16                    # DMA/exp chunk along T

_BASS_CACHE = {}


def _band(t):
    """Live CTC band [lo, hi] (inclusive states) at step t; lo forced even."""
    lo = max(0, 2 * t - 66)
    hi = min(S - 1, 2 * t + 1)
    return lo, hi


def _build_bass():
    if "nc" in _BASS_CACHE:
        return _BASS_CACHE["nc"]

    import concourse.bacc as bacc
    import concourse.mybir as mybir
    from concourse.tile import TileContext

    f32 = mybir.dt.float32
    bf16 = mybir.dt.bfloat16
    AF = mybir.ActivationFunctionType

    nc = bacc.Bacc(trn_type="TRN2")
    xd_d = nc.declare_dram_parameter("xd", [P, T, G, L], bf16, isOutput=False)
    xdm_d = nc.declare_dram_parameter("xdm", [P, T, G, L], bf16, isOutput=False)
    blkl_d = nc.declare_dram_parameter("blkl", [P, G, T], f32, isOutput=False)
    loss_d = nc.declare_dram_parameter("loss", [P, G], f32, isOutput=True)

    with TileContext(nc) as tc:
        with tc.tile_pool(name="main", bufs=1) as pool:
            blkl = pool.tile([P, G, T], f32)
            nc.sync.dma_start(out=blkl[:], in_=blkl_d[:])

            xd_s = pool.tile([P, T, G, L], bf16)
            xdm_s = pool.tile([P, T, G, L], bf16)
            r = pool.tile([P, T, G, L], f32)
            r2 = pool.tile([P, T, G, L], f32)
            for c in range(T // TCH):
                sl = slice(c * TCH, (c + 1) * TCH)
                nc.sync.dma_start(out=xd_s[:, sl], in_=xd_d[:, sl])
                nc.scalar.activation(out=r[:, sl], in_=xd_s[:, sl], func=AF.Exp)
                nc.sync.dma_start(out=xdm_s[:, sl], in_=xdm_d[:, sl])
                nc.scalar.activation(out=r2[:, sl], in_=xdm_s[:, sl], func=AF.Exp)

            # alpha state: cols 0,1 stay zero; state s lives in col s+2
            a = pool.tile([P, G, S + 2], f32)
            b = pool.tile([P, G, S + 2], f32)
            vv = pool.tile([P, G, L], f32)
            nc.vector.memset(a[:], 0.0)
            nc.vector.memset(b[:], 0.0)
            # a~0[s=0] = 1; a~0[s=1] = r[t=0, j=0]
            nc.vector.memset(a[:, :, 2], 1.0)
            nc.scalar.copy(out=a[:, :, 3], in_=r[:, 0, :, 0])

            cur, nxt = a, b
            for t in range(1, T):
                lo, hi = _band(t)
                clo, chi = lo + 2, hi + 2
                ho = hi if hi % 2 == 1 else hi - 1   # top odd state
                jlo, jhi = lo // 2, (ho - 1) // 2    # inclusive label idx range
                nj = jhi - jlo + 1
                # nxt[s] = cur[s] + cur[s-1] over the band (both parities)
                nc.vector.tensor_add(
                    out=nxt[:, :, clo : chi + 1],
                    in0=cur[:, :, clo : chi + 1],
                    in1=cur[:, :, clo - 1 : chi],
                )
                # vv[j] = cur[s-2] * r2[t,j] at odd s=2j+1 (cur col 2j+1)
                nc.vector.tensor_mul(
                    out=vv[:, :, jlo : jhi + 1],
                    in0=cur[:, :, 2 * jlo + 1 : 2 * jhi + 2 : 2],
                    in1=r2[:, t, :, jlo : jhi + 1],
                )
                # odd lanes: *= r, += vv
                nc.vector.tensor_mul(
                    out=nxt[:, :, 2 * jlo + 3 : 2 * jhi + 4 : 2],
                    in0=nxt[:, :, 2 * jlo + 3 : 2 * jhi + 4 : 2],
                    in1=r[:, t, :, jlo : jhi + 1],
                )
                nc.vector.tensor_add(
                    out=nxt[:, :, 2 * jlo + 3 : 2 * jhi + 4 : 2],
                    in0=nxt[:, :, 2 * jlo + 3 : 2 * jhi + 4 : 2],
                    in1=vv[:, :, jlo : jhi + 1],
                )
                cur, nxt = nxt, cur

            # loss = -( log(a~[S-1] + a~[S-2]) + sum_t x_blank[t] )
            stot = pool.tile([P, G], f32)
            nc.vector.tensor_add(
                out=stot[:], in0=cur[:, :, S + 1], in1=cur[:, :, S]
            )
            # a~ reaches ~1e22, beyond the ACT Ln table's accurate range
            # [1e-15, 1e15]; pre-scale by 2^-32 inside the activation and add
            # the 32*ln2 back in the final negate's bias.
            lg = pool.tile([P, G], f32)
            nc.scalar.activation(
                out=lg[:], in_=stot[:], func=AF.Ln, scale=float(2.0**-32)
            )
            bsum = pool.tile([P, G, 1], f32)
            nc.vector.reduce_sum(out=bsum[:], in_=blkl[:], axis=mybir.AxisListType.X)
            tot = pool.tile([P, G], f32)
            nc.vector.tensor_add(out=tot[:], in0=lg[:], in1=bsum[:, :, 0])
            neg = pool.tile([P, G], f32)
            nc.scalar.activation(
                out=neg[:],
                in_=tot[:],
                func=AF.Copy,
                scale=-1.0,
                bias=float(-32.0 * np.log(2.0)),
            )
            nc.sync.dma_start(out=loss_d[:], in_=neg[:])

    nc.finalize()
    _BASS_CACHE["nc"] = nc
    return nc


def host_prep(input, target, input_length, target_length):
    """Build the 8 per-core input maps."""
    import ml_dtypes

    inp = np.asarray(input, dtype=np.float32)
    target = np.asarray(target, dtype=np.int32)
    tl = np.asarray(target_length, dtype=np.int64)

    # reference's buggy padding: start_i = target_length[i-1] if i>0 else 0,
    # clamped like jax.lax.dynamic_slice
    starts = np.zeros(N, np.int64)
    starts[1:] = tl[: N - 1]
    starts = np.clip(starts, 0, len(target) - L)
    lab = target[starts[:, None] + np.arange(L)]  # [N, L] int32
    z = np.ones((N, L), np.float32)
    z[:, 1:] = (lab[:, 1:] != lab[:, :-1]).astype(np.float32)

    x_nct = inp.transpose(1, 2, 0)  # [N, C, T] view
    xs = np.take_along_axis(x_nct, lab[:, :, None].astype(np.int64), axis=1)
    blk = x_nct[:, 0, :]                       # [N, T]
    xd = xs - blk[:, None, :]                  # [N, L, T]
    xdm = np.where(z[:, :, None] == 0.0, np.float32(-1e30), xd)
    xd = np.ascontiguousarray(xd.transpose(0, 2, 1)).astype(ml_dtypes.bfloat16)
    xdm = np.ascontiguousarray(xdm.transpose(0, 2, 1)).astype(ml_dtypes.bfloat16)

    in_maps = []
    for core in range(NCORES):
        sl = slice(core * NLOC, (core + 1) * NLOC)
        xd_c = xd[sl].reshape(G, P, T, L).transpose(1, 2, 0, 3)
        xdm_c = xdm[sl].reshape(G, P, T, L).transpose(1, 2, 0, 3)
        blk_c = blk[sl].reshape(G, P, T).transpose(1, 0, 2)
        in_maps.append(
            {
                "xd": np.ascontiguousarray(xd_c),
                "xdm": np.ascontiguousarray(xdm_c),
                "blkl": np.ascontiguousarray(blk_c),
            }
        )
    return in_maps


def kernel(input, target, input_length, target_length):
    from concourse.bass_utils import run_bass_kernel_spmd

    nc = _build_bass()
    in_maps = host_prep(input, target, input_length, target_length)
    res = run_bass_kernel_spmd(nc, in_maps, list(range(NCORES)))
    total = 0.0
    for core in range(NCORES):
        total += float(np.asarray(res.results[core]["loss"], dtype=np.float64).sum())
    return np.float32(total)


# revision 5
# speedup vs baseline: 1.6205x; 1.1942x over previous
"""CTC-style loss (nn_CTCFormal) on 8 Trainium2 NeuronCores.

Pure data parallel over batch N=4096 -> 512 samples/core, laid out as
[P=128 partitions, G=4 groups].

Formulation: the alpha recurrence is rescaled by the per-step blank
probability.  With a~[t,s] = alpha[t,s] / prod_{tau<=t} y_blank(tau):

  even s (blanks):  a~[t,s] = a~[t-1,s] + a~[t-1,s-1]           (no multiply)
  odd  s (labels):  a~[t,s] = (a~[t-1,s] + a~[t-1,s-1]) * r[t,j]
                              + a~[t-1,s-2] * r2[t,j]
  loss = -( log(a~[T-1,S-1] + a~[T-1,S-2]) + sum_t x_blank[t] )

where r = exp(x_lab - x_blank) and r2 = z * r with z the CTC skip mask
(z=0 when lab[j]==lab[j-1]); r2 comes from exp() of a -1e30-masked copy.
The blank product becomes a log-space reduce_sum of raw blank logits, so
the blank multiply disappears from the inner loop entirely (4 DVE ops/step
instead of 5, and one of them is the only op that reads r).

The alpha DP is banded: at step t only states s in [max(0,2t-66),
min(62,2t+1)] are live (states outside can't be on any path from
(0,{0,1}) to (T-1,{S-1,S-2})), so ops slice just the live band (~53% of
the elements, exact).

Host prep replicates the reference's (buggy) target padding, gathers the
per-sample label-class logit rows (index-only data movement; this
environment's SWDGE gather ucode faults), subtracts the blank row, and
ships bf16 [P, T, G, L] time-major tensors so each step's r slice is
contiguous.

Device (per core): chunked DMAs + ACT exp produce r/r2 overlapped with
the DVE recurrence; final -log and blank reduce_sum close it out.  The
host sums the 8x512 partials (the all-reduce of the scalar loss sum).
"""

import numpy as np

T, N, C = 64, 4096, 128
L = 31           # labels per sample
S = 2 * L + 1    # 63 padded states
NCORES = 8
NLOC = N // NCORES          # 512 samples per core
G = NLOC // 128             # 4 groups of 128 samples (partition dim)
P = 128
TCH = 8                     # DMA/exp chunk along T

_BASS_CACHE = {}


def _band(t):
    """Live CTC band [lo, hi] (inclusive states) at step t; lo forced even."""
    lo = max(0, 2 * t - 66)
    hi = min(S - 1, 2 * t + 1)
    return lo, hi


def _build_bass():
    if "nc" in _BASS_CACHE:
        return _BASS_CACHE["nc"]

    import concourse.bacc as bacc
    import concourse.mybir as mybir
    from concourse.tile import TileContext

    f32 = mybir.dt.float32
    bf16 = mybir.dt.bfloat16
    AF = mybir.ActivationFunctionType

    nc = bacc.Bacc(trn_type="TRN2")
    xd_d = nc.declare_dram_parameter("xd", [P, T, G, L], bf16, isOutput=False)
    xdm_d = nc.declare_dram_parameter("xdm", [P, T, G, L], bf16, isOutput=False)
    blkl_d = nc.declare_dram_parameter("blkl", [P, G, T], f32, isOutput=False)
    loss_d = nc.declare_dram_parameter("loss", [P, G], f32, isOutput=True)

    with TileContext(nc) as tc:
        with tc.tile_pool(name="main", bufs=1) as pool:
            blkl = pool.tile([P, G, T], f32)
            nc.sync.dma_start(out=blkl[:], in_=blkl_d[:])

            # alpha state: cols 0,1 stay zero; state s lives in col s+2
            a = pool.tile([P, G, S + 2], f32)
            b = pool.tile([P, G, S + 2], f32)
            vv = pool.tile([P, G, L], f32)
            nc.vector.memset(a[:], 0.0)
            nc.vector.memset(b[:], 0.0)
            nc.vector.memset(a[:, :, 2], 1.0)

            xd_s = pool.tile([P, T, G, L], bf16)
            xdm_s = pool.tile([P, T, G, L], bf16)
            r = pool.tile([P, T, G, L], f32)
            r2 = pool.tile([P, T, G, L], f32)
            for c in range(T // TCH):
                sl = slice(c * TCH, (c + 1) * TCH)
                nc.sync.dma_start(out=xd_s[:, sl], in_=xd_d[:, sl])
                nc.scalar.activation(out=r[:, sl], in_=xd_s[:, sl], func=AF.Exp)
                nc.sync.dma_start(out=xdm_s[:, sl], in_=xdm_d[:, sl])
                nc.scalar.activation(out=r2[:, sl], in_=xdm_s[:, sl], func=AF.Exp)
                if c == 0:
                    # a~0[s=1] = r[t=0, j=0]; emitted here so ACT runs it
                    # before the later chunks' exps (ACT executes in order)
                    nc.scalar.copy(out=a[:, :, 3], in_=r[:, 0, :, 0])

            cur, nxt = a, b
            for t in range(1, T):
                lo, hi = _band(t)
                clo, chi = lo + 2, hi + 2
                ho = hi if hi % 2 == 1 else hi - 1   # top odd state
                jlo, jhi = lo // 2, (ho - 1) // 2    # inclusive label idx range
                nj = jhi - jlo + 1
                # nxt[s] = cur[s] + cur[s-1] over the band (both parities)
                nc.vector.tensor_add(
                    out=nxt[:, :, clo : chi + 1],
                    in0=cur[:, :, clo : chi + 1],
                    in1=cur[:, :, clo - 1 : chi],
                )
                # vv[j] = cur[s-2] * r2[t,j] at odd s=2j+1 (cur col 2j+1)
                nc.vector.tensor_mul(
                    out=vv[:, :, jlo : jhi + 1],
                    in0=cur[:, :, 2 * jlo + 1 : 2 * jhi + 2 : 2],
                    in1=r2[:, t, :, jlo : jhi + 1],
                )
                # odd lanes: *= r, += vv
                nc.vector.tensor_mul(
                    out=nxt[:, :, 2 * jlo + 3 : 2 * jhi + 4 : 2],
                    in0=nxt[:, :, 2 * jlo + 3 : 2 * jhi + 4 : 2],
                    in1=r[:, t, :, jlo : jhi + 1],
                )
                nc.vector.tensor_add(
                    out=nxt[:, :, 2 * jlo + 3 : 2 * jhi + 4 : 2],
                    in0=nxt[:, :, 2 * jlo + 3 : 2 * jhi + 4 : 2],
                    in1=vv[:, :, jlo : jhi + 1],
                )
                cur, nxt = nxt, cur

            # loss = -( log(a~[S-1] + a~[S-2]) + sum_t x_blank[t] )
            stot = pool.tile([P, G], f32)
            nc.vector.tensor_add(
                out=stot[:], in0=cur[:, :, S + 1], in1=cur[:, :, S]
            )
            # a~ reaches ~1e22, beyond the ACT Ln table's accurate range
            # [1e-15, 1e15]; pre-scale by 2^-32 inside the activation and add
            # the 32*ln2 back in the final negate's bias.
            lg = pool.tile([P, G], f32)
            nc.scalar.activation(
                out=lg[:], in_=stot[:], func=AF.Ln, scale=float(2.0**-32)
            )
            bsum = pool.tile([P, G, 1], f32)
            nc.vector.reduce_sum(out=bsum[:], in_=blkl[:], axis=mybir.AxisListType.X)
            tot = pool.tile([P, G], f32)
            nc.vector.tensor_add(out=tot[:], in0=lg[:], in1=bsum[:, :, 0])
            neg = pool.tile([P, G], f32)
            nc.scalar.activation(
                out=neg[:],
                in_=tot[:],
                func=AF.Copy,
                scale=-1.0,
                bias=float(-32.0 * np.log(2.0)),
            )
            nc.sync.dma_start(out=loss_d[:], in_=neg[:])

    nc.finalize()
    _BASS_CACHE["nc"] = nc
    return nc


def host_prep(input, target, input_length, target_length):
    """Build the 8 per-core input maps."""
    import ml_dtypes

    inp = np.asarray(input, dtype=np.float32)
    target = np.asarray(target, dtype=np.int32)
    tl = np.asarray(target_length, dtype=np.int64)

    # reference's buggy padding: start_i = target_length[i-1] if i>0 else 0,
    # clamped like jax.lax.dynamic_slice
    starts = np.zeros(N, np.int64)
    starts[1:] = tl[: N - 1]
    starts = np.clip(starts, 0, len(target) - L)
    lab = target[starts[:, None] + np.arange(L)]  # [N, L] int32
    z = np.ones((N, L), np.float32)
    z[:, 1:] = (lab[:, 1:] != lab[:, :-1]).astype(np.float32)

    x_nct = inp.transpose(1, 2, 0)  # [N, C, T] view
    xs = np.take_along_axis(x_nct, lab[:, :, None].astype(np.int64), axis=1)
    blk = x_nct[:, 0, :]                       # [N, T]
    xd = xs - blk[:, None, :]                  # [N, L, T]
    xdm = np.where(z[:, :, None] == 0.0, np.float32(-1e30), xd)
    xd = np.ascontiguousarray(xd.transpose(0, 2, 1)).astype(ml_dtypes.bfloat16)
    xdm = np.ascontiguousarray(xdm.transpose(0, 2, 1)).astype(ml_dtypes.bfloat16)

    in_maps = []
    for core in range(NCORES):
        sl = slice(core * NLOC, (core + 1) * NLOC)
        xd_c = xd[sl].reshape(G, P, T, L).transpose(1, 2, 0, 3)
        xdm_c = xdm[sl].reshape(G, P, T, L).transpose(1, 2, 0, 3)
        blk_c = blk[sl].reshape(G, P, T).transpose(1, 0, 2)
        in_maps.append(
            {
                "xd": np.ascontiguousarray(xd_c),
                "xdm": np.ascontiguousarray(xdm_c),
                "blkl": np.ascontiguousarray(blk_c),
            }
        )
    return in_maps


def kernel(input, target, input_length, target_length):
    from concourse.bass_utils import run_bass_kernel_spmd

    nc = _build_bass()
    in_maps = host_prep(input, target, input_length, target_length)
    res = run_bass_kernel_spmd(nc, in_maps, list(range(NCORES)))
    total = 0.0
    for core in range(NCORES):
        total += float(np.asarray(res.results[core]["loss"], dtype=np.float64).sum())
    return np.float32(total)
